# revision 14
# baseline (speedup 1.0000x reference)
"""DiffusionNet block on 8 Trainium2 NeuronCores.

Sharding: one graph per core (B=8). Per core, the whole pipeline runs in a
transposed dataflow (channels on partitions) so BatchNorm scale/bias are
per-partition ACT ops:

  to_basis (own graph) -> AllGather x_spec -> diffusion coefs D_g ->
  from_basis (own) -> spmm gX,gY via dma_gather(evec rows, fp16) +
  val-indicator matmuls (H^T per source graph in PSUM) + D_g contraction ->
  complex A matmuls -> tanh grad features -> 3-layer MLP with global-BN
  (partial sums + AllReduce) -> residual -> output (transposed; host undoes).

The sparse operators are NOT block-diagonal per graph: rows/cols are random
over the full vertex set.  Each output row-block (owned by one core) gathers
eigenvector rows of source vertices from all graphs (int16 indices into two
32768-row tables), and the per-128-nnz indicator matmul performs the
val-scaling and row-scatter in one PE pass.

All matmuls run fp16 operands with fp32 PSUM accumulation; statistics and
normalization math are fp32.
"""
import os
import sys
import numpy as np

sys.path.insert(0, '/opt/trn_rl_repo')

B, V, K, C = 8, 8192, 128, 256
N = B * V
NCORES = 8
P = 128
RG = 256            # rows per output group
NG = V // RG        # 32 groups per core
EPS = 1e-5

_LAST = {}          # debug/timing info from the most recent run


# ----------------------------------------------------------------- host prep

def _wrap_idx(idx):
    """int16 gather index layout: idx[i] at [i%16, i//16], tiled to 128 parts."""
    w = idx.reshape(-1, 16).T.astype(np.int16)
    return np.tile(w, (8, 1))


def _prep_sparse(rows, cols, vals):
    """Sort/pad one sparse operator into the per-core gather + indicator layout.

    Returns (schedule, per_core), where schedule (shared across cores) is
      chunks[G][g]  - number of 128-nnz chunks for each (row-group, src-graph)
    and per_core[c] has idx0/idx1 (wrapped int16 gather indices, per table) and
    rloc/rval ((128, NCHUNK) f32 indicator scalars in gathered-chunk order).
    """
    core = (rows >> 13).astype(np.int64)
    rloc_core = rows & (V - 1)
    grp = rloc_core >> 8
    rin = (rloc_core & (RG - 1)).astype(np.float32)
    g = (cols >> 13).astype(np.int64)
    tidx = ((g & 3) << 13 | (cols & (V - 1))).astype(np.int16)

    key = (core * NG + grp) * B + g
    order = np.argsort(key, kind='stable')
    key_s = key[order]
    tidx_s = tidx[order]
    rin_s = rin[order]
    val_s = vals[order].astype(np.float32)

    counts = np.bincount(key_s, minlength=NCORES * NG * B).reshape(NCORES, NG, B)
    chunks = np.maximum(1, -(-counts.max(axis=0) // P))     # (NG, B) shared
    starts = np.concatenate([[0], np.cumsum(counts.reshape(-1))]).astype(np.int64)

    nchunk = int(chunks.sum())
    per_core = []
    for c in range(NCORES):
        idx_pad = np.zeros(nchunk * P, np.int16)
        r_pad = np.zeros(nchunk * P, np.float32)
        v_pad = np.zeros(nchunk * P, np.float32)
        off = 0
        for G in range(NG):
            for gg in range(B):
                kk = (c * NG + G) * B + gg
                s, e = starts[kk], starts[kk + 1]
                n = e - s
                idx_pad[off:off + n] = tidx_s[s:e]
                r_pad[off:off + n] = rin_s[s:e]
                v_pad[off:off + n] = val_s[s:e]
                off += int(chunks[G, gg]) * P
        # table split: first 4 graphs -> table0, last 4 -> table1, per group
        idx0_parts, idx1_parts = [], []
        off = 0
        for G in range(NG):
            n0 = int(chunks[G, :4].sum()) * P
            n1 = int(chunks[G, 4:].sum()) * P
            idx0_parts.append(idx_pad[off:off + n0])
            idx1_parts.append(idx_pad[off + n0:off + n0 + n1])
            off += n0 + n1
        per_core.append(dict(
            idx0=_wrap_idx(np.concatenate(idx0_parts)),
            idx1=_wrap_idx(np.concatenate(idx1_parts)),
            rloc=r_pad.reshape(-1, P).T.copy(),
            rval=v_pad.reshape(-1, P).T.copy(),
        ))
    return chunks, per_core


def _prep(inputs):
    f16 = np.float16
    x = np.asarray(inputs['x'], np.float32)
    mass = np.asarray(inputs['mass'], np.float32)
    evals = np.asarray(inputs['evals'], np.float32)
    evecs = np.asarray(inputs['evecs'], np.float32)
    t = np.maximum(np.asarray(inputs['diffusion_time'], np.float32), 1e-8)
    A_re = np.asarray(inputs['A_re'], np.float32)
    A_im = np.asarray(inputs['A_im'], np.float32)

    evecs16 = evecs.astype(f16)
    etab0 = np.ascontiguousarray(evecs16[:N // 2])
    etab1 = np.ascontiguousarray(evecs16[N // 2:])

    def blocks_T(M, nch, nh):
        # lhsT layout (p, ch, h, q) = M[ch*128+p, h*128+q]
        return np.ascontiguousarray(
            M.reshape(nch, P, nh, P).transpose(1, 0, 2, 3)).astype(f16)

    w0b = blocks_T(np.asarray(inputs['W0'], np.float32), 6, 2)
    w1b = blocks_T(np.asarray(inputs['W1'], np.float32), 2, 2)
    w2b = blocks_T(np.asarray(inputs['W2'], np.float32), 2, 2)
    areb = blocks_T(A_re.T, 2, 2)            # lhsT[c, c'] = A_re[c', c]
    aimb = blocks_T(A_im.T, 2, 2)
    naimb = blocks_T(-A_im.T, 2, 2)

    gbe = np.stack([np.asarray(inputs[k], np.float32).reshape(2, P).T
                    for k in ('g0', 'g1', 'g2', 'be0', 'be1', 'be2')], axis=1)
    # gbe: (128, 6, 2) -> [p, which, half]

    iota16 = np.tile(np.arange(RG, dtype=np.float32), (P, 1)).astype(f16)
    t_b = np.tile(t, (P, 1)).astype(np.float32)
    evals_kg = evals.reshape(B, P).T.copy()          # (128, 8)

    sch_gx, pc_gx = _prep_sparse(np.asarray(inputs['gx_rows']),
                                 np.asarray(inputs['gx_cols']),
                                 np.asarray(inputs['gx_vals']))
    sch_gy, pc_gy = _prep_sparse(np.asarray(inputs['gy_rows']),
                                 np.asarray(inputs['gy_cols']),
                                 np.asarray(inputs['gy_vals']))

    in_maps = []
    for c in range(NCORES):
        sl = slice(c * V, (c + 1) * V)
        xc = x[sl]
        xT16 = np.ascontiguousarray(
            xc.reshape(V, 2, P).transpose(2, 1, 0)).astype(f16)   # (128,2,8192)
        xT32 = np.ascontiguousarray(
            xc.reshape(V, 2, P).transpose(2, 1, 0)).astype(np.float32)
        evT16 = np.ascontiguousarray(evecs[sl].T).astype(f16)     # (128,8192)
        m = dict(
            x16=np.ascontiguousarray(
                xc.reshape(64, P, C).transpose(1, 0, 2)).astype(f16),
            xT16=xT16, xT32=xT32,
            ev16=np.ascontiguousarray(
                evecs16[sl].reshape(64, P, K).transpose(1, 0, 2)),
            evT16=evT16,
            mass=np.ascontiguousarray(mass[sl].reshape(64, P).T),
            evals_own=evals[c * P:(c + 1) * P].reshape(P, 1).copy(),
            evals_kg=evals_kg, t_b=t_b, iota16=iota16,
            etab0=etab0, etab1=etab1,
            w0b=w0b, w1b=w1b, w2b=w2b,
            areb=areb, aimb=aimb, naimb=naimb, gbe=gbe,
            gx_idx0=pc_gx[c]['idx0'], gx_idx1=pc_gx[c]['idx1'],
            gx_rloc=pc_gx[c]['rloc'], gx_rval=pc_gx[c]['rval'],
            gy_idx0=pc_gy[c]['idx0'], gy_idx1=pc_gy[c]['idx1'],
            gy_rloc=pc_gy[c]['rloc'], gy_rval=pc_gy[c]['rval'],
        )
        in_maps.append(m)
    return (sch_gx, sch_gy), in_maps


# -------------------------------------------------------------- bass program

def _build(schedules):
    import concourse.bass as bass
    import concourse.mybir as mybir
    import concourse.tile as tile
    from concourse import bacc

    F32, F16, I16 = mybir.dt.float32, mybir.dt.float16, mybir.dt.int16
    AF = mybir.ActivationFunctionType
    OP = mybir.AluOpType
    sch_gx, sch_gy = schedules
    nch = {m: int(s.sum()) for m, s in (('gx', sch_gx), ('gy', sch_gy))}
    ni0 = {'gx': int(sch_gx[:, :4].sum()) * P, 'gy': int(sch_gy[:, :4].sum()) * P}
    ni1 = {'gx': int(sch_gx[:, 4:].sum()) * P, 'gy': int(sch_gy[:, 4:].sum()) * P}

    kphase = int(os.environ.get('KPHASE', '99'))
    knocc = bool(int(os.environ.get('KNOCC', '0')))
    ncores_prog = 1 if knocc else NCORES
    kpart = int(os.environ.get('KPART', '99'))
    kglim = int(os.environ.get('KGLIM', str(NG)))
    nc = bacc.Bacc("TRN2", target_bir_lowering=False, debug=False,
                   num_devices=ncores_prog)

    # ---- dram I/O
    d = {}
    def din(name, shape, dt):
        d[name] = nc.dram_tensor(name, list(shape), dt, kind="ExternalInput")
    din('x16', (P, 64, C), F16); din('xT16', (P, 2, V), F16); din('xT32', (P, 2, V), F32)
    din('ev16', (P, 64, K), F16); din('evT16', (P, V), F16)
    din('mass', (P, 64), F32); din('evals_own', (P, 1), F32)
    din('evals_kg', (P, B), F32); din('t_b', (P, C), F32); din('iota16', (P, RG), F16)
    din('etab0', (N // 2, K), F16); din('etab1', (N // 2, K), F16)
    din('w0b', (P, 6, 2, P), F16); din('w1b', (P, 2, 2, P), F16)
    din('w2b', (P, 2, 2, P), F16)
    din('areb', (P, 2, 2, P), F16); din('aimb', (P, 2, 2, P), F16)
    din('naimb', (P, 2, 2, P), F16); din('gbe', (P, 6, 2), F32)
    for m in ('gx', 'gy'):
        din(f'{m}_idx0', (P, ni0[m] // 16), I16)
        din(f'{m}_idx1', (P, ni1[m] // 16), I16)
        din(f'{m}_rloc', (P, nch[m]), F32)
        din(f'{m}_rval', (P, nch[m]), F32)
    outT = nc.dram_tensor('outT', [P, 2, V], F32, kind="ExternalOutput")

    ag_in = nc.dram_tensor('ag_in', [P, C], F32)
    ag_out = nc.dram_tensor('ag_out', [B, P, C], F32, addr_space="Shared")
    ar_in = [nc.dram_tensor(f'ar_in{i}', [P, 4], F32) for i in range(3)]
    ar_out = [nc.dram_tensor(f'ar_out{i}', [P, 4], F32, addr_space="Shared")
              for i in range(3)]

    RG_ALL = list(range(NCORES))

    with tile.TileContext(nc) as tc:
      with (
        tc.tile_pool(name="cst", bufs=1) as cst,
        tc.tile_pool(name="big", bufs=1) as big,
        tc.tile_pool(name="hbig", bufs=2) as hbigp,
        tc.tile_pool(name="st", bufs=3) as st,
        tc.tile_pool(name="s2", bufs=2) as s2,
        tc.tile_pool(name="s1", bufs=1) as s1,
        tc.tile_pool(name="ind", bufs=6) as indp,
        tc.tile_pool(name="gath", bufs=2) as gathp,
        tc.tile_pool(name="ht", bufs=1) as htp,
        tc.tile_pool(name="ps512", bufs=4, space="PSUM") as ps512,
        tc.tile_pool(name="psht", bufs=2, space="PSUM") as psht,
      ):
        # ---------------- constants
        iot = cst.tile([P, RG], F16); nc.sync.dma_start(iot[:], d['iota16'][:, :])
        tb = cst.tile([P, C], F32); nc.sync.dma_start(tb[:], d['t_b'][:, :])
        ekg = cst.tile([P, B], F32); nc.sync.dma_start(ekg[:], d['evals_kg'][:, :])
        eo = cst.tile([P, 1], F32); nc.sync.dma_start(eo[:], d['evals_own'][:, :])
        msb = cst.tile([P, 64], F32); nc.sync.dma_start(msb[:], d['mass'][:, :])
        w0 = cst.tile([P, 6, 2, P], F16); nc.sync.dma_start(w0[:], d['w0b'][:, :, :, :])
        w1 = cst.tile([P, 2, 2, P], F16); nc.sync.dma_start(w1[:], d['w1b'][:, :, :, :])
        w2 = cst.tile([P, 2, 2, P], F16); nc.sync.dma_start(w2[:], d['w2b'][:, :, :, :])
        are = cst.tile([P, 2, 2, P], F16); nc.sync.dma_start(are[:], d['areb'][:, :, :, :])
        aim = cst.tile([P, 2, 2, P], F16); nc.sync.dma_start(aim[:], d['aimb'][:, :, :, :])
        nai = cst.tile([P, 2, 2, P], F16); nc.sync.dma_start(nai[:], d['naimb'][:, :, :, :])
        gbe = cst.tile([P, 6, 2], F32); nc.sync.dma_start(gbe[:], d['gbe'][:, :, :])
        epsb = cst.tile([P, 1], F32); nc.vector.memset(epsb[:], EPS)

        # ---------------- phase 1: to_basis (own graph)
        xs_ps = ps512.tile([P, 512], F32, tag="p512")
        for vb in range(8):
            xt8 = s2.tile([P, 8, C], F16, tag="p1x")
            nc.sync.dma_start(xt8[:], d['x16'][:, vb * 8:(vb + 1) * 8, :])
            et8 = s2.tile([P, 8, K], F16, tag="p1e")
            nc.sync.dma_start(et8[:], d['ev16'][:, vb * 8:(vb + 1) * 8, :])
            for vj in range(8):
                vc = vb * 8 + vj
                nc.vector.tensor_scalar(xt8[:, vj, :], xt8[:, vj, :],
                                        msb[:, vc:vc + 1], None, OP.mult)
                nc.tensor.matmul(xs_ps[:, :C], lhsT=et8[:, vj, :], rhs=xt8[:, vj, :],
                                 start=(vc == 0), stop=(vc == 63))
        xspec = big.tile([P, C], F32)
        nc.scalar.copy(xspec[:], xs_ps[:, :C])
        nc.sync.dma_start(ag_in[:, :], xspec[:])
        if knocc:
            for gg in range(B):
                nc.sync.dma_start(ag_out[gg, :, :], ag_in[:, :])
        else:
            nc.gpsimd.collective_compute(
                "AllGather", OP.bypass, replica_groups=[RG_ALL],
                ins=[ag_in[:, :]], outs=[ag_out[:, :, :]])

        # own-graph diffusion spectrum (independent of allgather)
        z = s2.tile([P, C], F32, tag="z")
        nc.vector.tensor_scalar(z[:], tb[:], eo[:, :1], None, OP.mult)
        cf = s2.tile([P, C], F32, tag="cf")
        nc.scalar.activation(cf[:], z[:], AF.Exp, scale=-1.0)
        Down = big.tile([P, C], F16)
        nc.vector.tensor_tensor(Down[:], cf[:], xspec[:], OP.mult)

        # all-graph spectra D16
        D16 = big.tile([P, B, C], F16)
        for gg in range(B):
            xs_g = s2.tile([P, C], F32, tag="xsg")
            nc.sync.dma_start(xs_g[:], ag_out[gg, :, :])
            z = s2.tile([P, C], F32, tag="z")
            nc.vector.tensor_scalar(z[:], tb[:], ekg[:, gg:gg + 1], None, OP.mult)
            cf = s2.tile([P, C], F32, tag="cf")
            nc.scalar.activation(cf[:], z[:], AF.Exp, scale=-1.0)
            nc.vector.tensor_tensor(D16[:, gg, :], cf[:], xs_g[:], OP.mult)

        # ---------------- phase 3: from_basis -> xdT16 (own graph, transposed)
        xdT = big.tile([P, 2, V], F16)
        for Gb in (range(4) if kphase >= 2 else []):
            evTg = s1.tile([P, 8, RG], F16, tag="evTg")
            nc.sync.dma_start(evTg[:], d['evT16'][:, Gb * 2048:(Gb + 1) * 2048]
                              .rearrange("p (o r) -> p o r", r=RG))
            for Gj in range(8):
                G = Gb * 8 + Gj
                xd_ps = ps512.tile([P, 512], F32, tag="p512")
                for h in range(2):
                    nc.tensor.matmul(xd_ps[:, h * RG:(h + 1) * RG],
                                     lhsT=Down[:, h * P:(h + 1) * P],
                                     rhs=evTg[:, Gj, :], start=True, stop=True)
                for h in range(2):
                    nc.scalar.copy(xdT[:, h, G * RG:(G + 1) * RG],
                                   xd_ps[:, h * RG:(h + 1) * RG])

        # ---------------- phase 4: spmm + A + tanh + W0, streamed per group
        h0T = hbigp.tile([P, 2, V], F16, tag="hb")
        sumP = st.tile([P, 2, 2, NG], F32, tag="sums")
        # layout: sumP[:, 0, h, G] = sum, sumP[:, 1, h, G] = sumsq  (layer 0)
        schs = {'gx': sch_gx, 'gy': sch_gy}
        cum0 = {m: np.concatenate([[0], np.cumsum(schs[m][:, :4].sum(axis=1))]) * P
                for m in ('gx', 'gy')}
        cum1 = {m: np.concatenate([[0], np.cumsum(schs[m][:, 4:].sum(axis=1))]) * P
                for m in ('gx', 'gy')}
        cumc = {m: np.concatenate([[0], np.cumsum(schs[m].sum(axis=1))])
                for m in ('gx', 'gy')}

        _xt4_cache = [None]
        for G in (range(min(NG, kglim)) if kphase >= 3 else []):
            gxyT = {}
            for m in ('gx', 'gy'):
                sch = schs[m]
                cG = int(sch[G].sum())
                gt = gathp.tile([P, cG, K], F16, tag="g")
                for half, (nic, dname) in enumerate(
                        [(cum0[m], f'{m}_idx0'), (cum1[m], f'{m}_idx1')]):
                    a, b_ = int(nic[G]), int(nic[G + 1])
                    niG = b_ - a
                    if niG == 0:
                        continue
                    ix = s2.tile([P, niG // 16], I16, tag=f"ix{half}")
                    nc.sync.dma_start(ix[:], d[dname][:, a // 16:b_ // 16])
                    coff = 0 if half == 0 else int(sch[G, :4].sum())
                    nc.gpsimd.dma_gather(
                        gt[:, coff:coff + niG // P, :],
                        d['etab0' if half == 0 else 'etab1'][:, :],
                        ix[:], niG, niG, K, single_packet=False)
                rv = s2.tile([P, 2, cG], F32, tag="rv")
                c0 = int(cumc[m][G])
                nc.sync.dma_start(rv[:, 0, :], d[f'{m}_rloc'][:, c0:c0 + cG])
                nc.sync.dma_start(rv[:, 1, :], d[f'{m}_rval'][:, c0:c0 + cG])

                if kpart < 2:
                    continue
                ht16 = htp.tile([P, 2048], F16, tag="ht16")
                cc = 0
                for hb in range(2):
                    ht_ps = psht.tile([P, 1024], F32, tag="htps", name=f"htps{hb}")
                    for gj in range(4):
                        gg = hb * 4 + gj
                        ckn = int(sch[G, gg])
                        for k in range(ckn):
                            ind = indp.tile([P, RG], F16, tag="ind")
                            nc.vector.tensor_scalar(
                                ind[:], iot[:], rv[:, 0, cc:cc + 1],
                                rv[:, 1, cc:cc + 1], OP.is_equal, OP.mult)
                            nc.tensor.matmul(ht_ps[:, gj * RG:(gj + 1) * RG],
                                             lhsT=gt[:, cc, :], rhs=ind[:],
                                             start=(k == 0), stop=(k == ckn - 1))
                            cc += 1
                    nc.scalar.copy(ht16[:, hb * 1024:(hb + 1) * 1024], ht_ps[:])
                if kpart < 3:
                    continue
                gps = ps512.tile([P, 512], F32, tag="p512")
                for h in range(2):
                    for gg in range(B):
                        nc.tensor.matmul(gps[:, h * RG:(h + 1) * RG],
                                         lhsT=D16[:, gg, h * P:(h + 1) * P],
                                         rhs=ht16[:, gg * RG:(gg + 1) * RG],
                                         start=(gg == 0), stop=(gg == B - 1))
                gxyT[m] = s2.tile([P, 2, RG], F16, tag=f"gt_{m}", name=f"gt_{m}")
                nc.scalar.copy(gxyT[m][:], gps[:])

            # complex A matmuls (transposed): bre^T, bim^T in PSUM
            if kpart < 4:
                continue
            bre = ps512.tile([P, 512], F32, tag="p512")
            bim = ps512.tile([P, 512], F32, tag="p512")
            for h in range(2):
                seq_re = [(are, gxyT['gx'], 0), (are, gxyT['gx'], 1),
                          (nai, gxyT['gy'], 0), (nai, gxyT['gy'], 1)]
                seq_im = [(are, gxyT['gy'], 0), (are, gxyT['gy'], 1),
                          (aim, gxyT['gx'], 0), (aim, gxyT['gx'], 1)]
                for ps, seq in ((bre, seq_re), (bim, seq_im)):
                    for i, (ab, gsrc, ch) in enumerate(seq):
                        nc.tensor.matmul(ps[:, h * RG:(h + 1) * RG],
                                         lhsT=ab[:, ch, h, :], rhs=gsrc[:, ch, :],
                                         start=(i == 0), stop=(i == 3))
            t1 = s2.tile([P, 2, RG], F32, tag="t1")
            t2 = s2.tile([P, 2, RG], F32, tag="t2")
            nc.vector.tensor_tensor(t1[:], gxyT['gx'][:], bre[:], OP.mult)
            nc.vector.tensor_tensor(t2[:], gxyT['gy'][:], bim[:], OP.mult)
            nc.vector.tensor_tensor(t1[:], t1[:], t2[:], OP.add)
            gf = s2.tile([P, 2, RG], F16, tag="gf")
            nc.scalar.activation(gf[:], t1[:], AF.Tanh)

            # W0 matmul: h0^T group slice
            if kpart < 5:
                continue
            if G % 4 == 0:
                xt4 = s2.tile([P, 2, 4, RG], F16, tag="xtw0", name="xt4")
                nc.sync.dma_start(xt4[:], d['xT16'][:, :, G * RG:(G + 4) * RG]
                                  .rearrange("p h (o r) -> p h o r", r=RG))
                _xt4_cache[0] = xt4
            xt = _xt4_cache[0][:, :, G % 4, :]
            h_ps = ps512.tile([P, 512], F32, tag="p512")
            rhs_list = [xt[:, 0, :], xt[:, 1, :],
                        xdT[:, 0, G * RG:(G + 1) * RG], xdT[:, 1, G * RG:(G + 1) * RG],
                        gf[:, 0, :], gf[:, 1, :]]
            for h in range(2):
                for ch in range(6):
                    nc.tensor.matmul(h_ps[:, h * RG:(h + 1) * RG],
                                     lhsT=w0[:, ch, h, :], rhs=rhs_list[ch],
                                     start=(ch == 0), stop=(ch == 5))
            for h in range(2):
                nc.scalar.activation(h0T[:, h, G * RG:(G + 1) * RG],
                                     h_ps[:, h * RG:(h + 1) * RG], AF.Copy,
                                     accum_out=sumP[:, 0, h, G:G + 1])
                sq = s2.tile([P, RG], F32, tag="sq")
                nc.scalar.activation(sq[:], h_ps[:, h * RG:(h + 1) * RG], AF.Square,
                                     accum_out=sumP[:, 1, h, G:G + 1])

        # ---------------- BN helper
        def bn_finalize(layer, sums_tile, relu):
            """AllReduce partial sums -> (scale, shift) (128, 2) tiles."""
            stt = st.tile([P, 4], F32, tag="stt")
            for h in range(2):
                nc.vector.tensor_reduce(stt[:, h:h + 1], sums_tile[:, 0, h, :],
                                        mybir.AxisListType.X, OP.add)
                nc.vector.tensor_reduce(stt[:, 2 + h:3 + h], sums_tile[:, 1, h, :],
                                        mybir.AxisListType.X, OP.add)
            nc.sync.dma_start(ar_in[layer][:, :], stt[:])
            if knocc:
                nc.sync.dma_start(ar_out[layer][:, :], ar_in[layer][:, :])
            else:
                nc.gpsimd.collective_compute(
                    "AllReduce", OP.add, replica_groups=[RG_ALL],
                    ins=[ar_in[layer][:, :]], outs=[ar_out[layer][:, :]])
            rst = st.tile([P, 4], F32, tag="rst")
            nc.sync.dma_start(rst[:], ar_out[layer][:, :])
            mean = st.tile([P, 2], F32, tag="mean")
            nc.vector.tensor_scalar(mean[:], rst[:, :2], 1.0 / N, None, OP.mult)
            var = st.tile([P, 2], F32, tag="var")
            nc.vector.tensor_scalar(var[:], rst[:, 2:], 1.0 / N, None, OP.mult)
            msq = st.tile([P, 2], F32, tag="msq")
            nc.vector.tensor_tensor(msq[:], mean[:], mean[:], OP.mult)
            nc.vector.tensor_tensor(var[:], var[:], msq[:], OP.subtract)
            sd = st.tile([P, 2], F32, tag="sd")
            nc.scalar.activation(sd[:], var[:], AF.Sqrt, bias=epsb[:, :1])
            rs = st.tile([P, 2], F32, tag="rs")
            nc.vector.reciprocal(rs[:], sd[:])
            sc = st.tile([P, 2], F32, tag=f"sc{layer}")
            nc.vector.tensor_tensor(sc[:], rs[:], gbe[:, layer, :], OP.mult)
            sh = st.tile([P, 2], F32, tag=f"sh{layer}")
            nc.vector.tensor_tensor(sh[:], mean[:], sc[:], OP.mult)
            nc.vector.tensor_tensor(sh[:], gbe[:, 3 + layer, :], sh[:], OP.subtract)
            return sc, sh

        if kphase >= 4:
            sc0, sh0 = bn_finalize(0, sumP, relu=True)
            for h in range(2):
                nc.scalar.activation(h0T[:, h, :], h0T[:, h, :], AF.Relu,
                                     bias=sh0[:, h:h + 1], scale=sc0[:, h:h + 1])

        # ---------------- W1
        h1T = hbigp.tile([P, 2, V], F16, tag="hb")
        sumP1 = st.tile([P, 2, 2, NG], F32, tag="sums1")
        for G in (range(NG) if kphase >= 5 else []):
            h_ps = ps512.tile([P, 512], F32, tag="p512")
            for h in range(2):
                for ch in range(2):
                    nc.tensor.matmul(h_ps[:, h * RG:(h + 1) * RG],
                                     lhsT=w1[:, ch, h, :],
                                     rhs=h0T[:, ch, G * RG:(G + 1) * RG],
                                     start=(ch == 0), stop=(ch == 1))
            for h in range(2):
                nc.scalar.activation(h1T[:, h, G * RG:(G + 1) * RG],
                                     h_ps[:, h * RG:(h + 1) * RG], AF.Copy,
                                     accum_out=sumP1[:, 0, h, G:G + 1])
                sq = s2.tile([P, RG], F32, tag="sq")
                nc.scalar.activation(sq[:], h_ps[:, h * RG:(h + 1) * RG], AF.Square,
                                     accum_out=sumP1[:, 1, h, G:G + 1])
        if kphase >= 5:
            sc1, sh1 = bn_finalize(1, sumP1, relu=True)
            for h in range(2):
                nc.scalar.activation(h1T[:, h, :], h1T[:, h, :], AF.Relu,
                                     bias=sh1[:, h:h + 1], scale=sc1[:, h:h + 1])

        # ---------------- W2
        h2T = hbigp.tile([P, 2, V], F16, tag="hb")
        sumP2 = st.tile([P, 2, 2, NG], F32, tag="sums2")
        for G in (range(NG) if kphase >= 6 else []):
            h_ps = ps512.tile([P, 512], F32, tag="p512")
            for h in range(2):
                for ch in range(2):
                    nc.tensor.matmul(h_ps[:, h * RG:(h + 1) * RG],
                                     lhsT=w2[:, ch, h, :],
                                     rhs=h1T[:, ch, G * RG:(G + 1) * RG],
                                     start=(ch == 0), stop=(ch == 1))
            for h in range(2):
                nc.scalar.activation(h2T[:, h, G * RG:(G + 1) * RG],
                                     h_ps[:, h * RG:(h + 1) * RG], AF.Copy,
                                     accum_out=sumP2[:, 0, h, G:G + 1])
                sq = s2.tile([P, RG], F32, tag="sq")
                nc.scalar.activation(sq[:], h_ps[:, h * RG:(h + 1) * RG], AF.Square,
                                     accum_out=sumP2[:, 1, h, G:G + 1])
        if kphase >= 6:
            sc2, sh2 = bn_finalize(2, sumP2, relu=False)

        # ---------------- BN2 apply + residual + store
        _xr4_cache = [None]
        for G in (range(NG) if kphase >= 6 else []):
            if G % 4 == 0:
                xr4 = s1.tile([P, 2, 4, RG], F32, tag="xr", name="xr4")
                nc.sync.dma_start(xr4[:], d['xT32'][:, :, G * RG:(G + 4) * RG]
                                  .rearrange("p h (o r) -> p h o r", r=RG))
                _xr4_cache[0] = xr4
            xr = _xr4_cache[0][:, :, G % 4, :]
            ot = s2.tile([P, 2, RG], F32, tag="ot")
            for h in range(2):
                nc.scalar.activation(ot[:, h, :], h2T[:, h, G * RG:(G + 1) * RG],
                                     AF.Identity, bias=sh2[:, h:h + 1],
                                     scale=sc2[:, h:h + 1])
            nc.vector.tensor_tensor(ot[:, 0, :], ot[:, 0, :], xr[:, 0, :], OP.add)
            nc.vector.tensor_tensor(ot[:, 1, :], ot[:, 1, :], xr[:, 1, :], OP.add)
            nc.sync.dma_start(outT[:, :, G * RG:(G + 1) * RG], ot[:])

    nc.compile()
    return nc


# ------------------------------------------------------------------- driver

def kernel(**inputs):
    from concourse.bass_utils import run_bass_kernel_spmd

    schedules, in_maps = _prep(inputs)
    nc = _build(schedules)
    trace = bool(int(os.environ.get('KERNEL_TRACE', '0')))
    res = run_bass_kernel_spmd(nc, in_maps, core_ids=list(range(NCORES)),
                               trace=trace, trace_cores=[0] if trace else None)
    _LAST['res'] = res
    out = np.empty((N, C), np.float32)
    for c in range(NCORES):
        oT = res.results[c]['outT']                      # (128, 2, 8192)
        out[c * V:(c + 1) * V] = oT.transpose(2, 1, 0).reshape(V, C)
    return out
